# revision 14
# baseline (speedup 1.0000x reference)
"""Trainium2 Bass kernel for the CriticSNN problem.

Reference computation (see problem statement):
  x = concat(state, action)               # [B, 144]
  h_in = x @ W_in.T + b_in                # [B, 512], constant over T steps
  T=8 steps of a 3-layer LIF chain (leaky integrate-and-fire,
  reset-by-subtraction, heaviside spikes), 2 hidden 512x512 matmuls/step
  out = (mean_t last-layer spikes) @ W_out.T + b_out   # [B, 1]

Strategy (data-parallel over 8 cores, B=16384 -> 2048/core):
  * Everything on-chip lives in [h, b] layout (h on partitions, batch on the
    free dim) so spikes feed the next matmul with no transposes; the host
    pre-transposes x once.
  * Spikes are held as signs s in {-1,+1} (bf16, exact). W @ spk01 with
    spk01=(s+1)/2 becomes (W/2) @ s + rowsum(W)/2; the rowsum folds into
    per-partition constants.
  * Membrane state is kept as p = mem - thr - k, where k = -c/(beta-1)
    cancels the per-step constant c, making the recurrence constant-free:
        u   = beta * p + pre          (DVE scalar_tensor_tensor)
        tau = -(thr/2) * s_prev       (DVE tensor_scalar, 2x mode)
        p'  = u + tau                 (Pool tensor_tensor)
        s'  = Sign(p' + k)            (ACT activation, per-partition bias)
    t=0 collapses to p0 = matmul + c0 (one tensor_scalar).
  * Hidden weights are split hi/lo bf16 (W/2 = hi + lo exactly to ~2^-18):
    8 accumulating bf16 matmuls per 128x512 output tile == fp32 accuracy at
    2x the speed of native fp32 matmul. The input matmul (non-binary x) runs
    in native fp32. Readout is 8 tiny M=1 bf16 matmuls per chunk.
  * Batch is processed in 4 chunks of 512 columns, two chunks resident at a
    time so the tensor engine always has an independent chunk to work on
    while the other chunk's LIF tail drains.

Host<->device transfer is the end-to-end bottleneck (the axon tunnel moves
~40 MB/s serialized), so:
  * Weights are NOT replicated to all 8 cores over the tunnel: each core
    receives 1/8 of a flat weight blob and the kernel reassembles the full
    blob on every core with an on-device AllGather (~2 MB, ~15 us over
    NeuronLink).  Collectives can't touch I/O tensors, so the shards bounce
    through internal DRAM tiles.
  * Only the per-core batch slice of x ships whole, in f32 -- fp16/bf16
    input quantization is NOT safe: the threshold dynamics amplify an h_in
    relative error of 1.8e-4 into ~0.24 output error (err ~ 17.5*sqrt(d)).
  * All small tensors (W_in.T shard, LIF constants, readout weights) are
    packed into two blob tensors so each core uploads 3 arrays total.
  * The jitted shard_map executable is built once and cached; re-tracing it
    per call costs ~450 ms.  Host prep writes straight into the global
    [8*rows, ...] arrays shard_map splits, so no per-call concatenation.
"""

import numpy as np
import ml_dtypes

B, S, A, H, LM1, T = 16384, 128, 16, 512, 2, 8
NCORES = 8
BC = B // NCORES            # batch per core (2048)
BT = 512                    # batch chunk (columns per matmul)
NCH = BC // BT              # chunks per core (4)
NJ = H // 128               # output partition tiles (4)
NK = H // 128               # contraction tiles (4)
SA = S + A                  # 144
WROWS = 2 * LM1 * H         # rows in the bf16 hidden-weight blob (2048)
WSH = WROWS // NCORES       # hidden-weight rows per core (256)
INSH = SA // NCORES         # W_in.T rows per core (18)

CNAMES = ["binc"] + [f"{p}_{li}" for li in range(3)
                     for p in ("c0", "beta", "nth2", "kk")]
NC_ = len(CNAMES)           # 13 constant tiles of [128, 4]
WBLR = WSH + 2              # wbl rows: weight shard + wouthi + woutlo
FBLR = INSH + NC_ + 1       # fbl rows: W_in.T shard + consts + oconst (32)

_F32 = np.float32
_BF16 = ml_dtypes.bfloat16


def _cols(v):
    """[512] -> [128, 4] (column j = rows of partition-tile j)."""
    return np.asarray(v, np.float64).astype(_F32).reshape(NJ, 128).T


def _prepare_host(inputs):
    """Build the global input arrays shard_map splits across cores.

    state/action ship untransposed and zero-copy ([B, feat] row-sharded is
    exactly the per-core layout); the kernel transposes on-device via the
    PE (identity matmul), which is idle during the host upload anyway.
    """
    out = {"st": np.ascontiguousarray(np.asarray(inputs["state"], _F32)),
           "ac": np.ascontiguousarray(np.asarray(inputs["action"], _F32))}
    out.update(_prepare_weights(inputs))
    return out


def _prepare_weights(inputs):
    """Build wbl/fbl weight blobs; cached while the weight inputs are
    unchanged (checked by content hash)."""
    import hashlib

    hsh = hashlib.blake2b(digest_size=16)
    wnames = ("W_in", "b_in", "beta_in", "thr_in", "W_h", "b_h", "beta_h",
              "thr_h", "W_out", "b_out")
    arrs = {k: np.ascontiguousarray(np.asarray(inputs[k], _F32))
            for k in wnames}
    for k in wnames:
        hsh.update(arrs[k])
    digest = hsh.digest()
    hit = _CACHED.get("wblobs")
    if hit is not None and hit[0] == digest:
        return hit[1]

    blobs = _build_weight_blobs(arrs)
    _CACHED["wblobs"] = (digest, blobs)
    return blobs


def _build_weight_blobs(inputs):
    W_in = np.asarray(inputs["W_in"], _F32)
    b_in = np.asarray(inputs["b_in"], _F32)
    W_h = np.asarray(inputs["W_h"], _F32)
    b_h = np.asarray(inputs["b_h"], _F32)
    W_out = np.asarray(inputs["W_out"], _F32)
    b_out = np.asarray(inputs["b_out"], _F32)
    betas = [np.asarray(inputs["beta_in"], _F32)] + \
            [np.asarray(inputs["beta_h"], _F32)[i] for i in range(LM1)]
    thrs = [np.asarray(inputs["thr_in"], _F32)] + \
           [np.asarray(inputs["thr_h"], _F32)[i] for i in range(LM1)]

    # --- wbl: [8*258, 512] bf16 ---
    # rows 0..255 per core: shard of [whi0; wlo0; whi1; wlo1] (each [H, H]
    # transposed, W/2 split hi/lo); row 256 = wouthi, row 257 = woutlo.
    wblob = np.empty((WROWS, H), _BF16)
    for li in range(LM1):
        w2 = W_h[li] * _F32(0.5)                    # exact in f32
        hi = w2.astype(_BF16)
        lo = (w2 - hi.astype(_F32)).astype(_BF16)
        wblob[(2 * li) * H:(2 * li + 1) * H] = hi.T
        wblob[(2 * li + 1) * H:(2 * li + 2) * H] = lo.T
    v = (W_out[0] * _F32(0.5)).astype(_F32)
    vhi = v.astype(_BF16)
    vlo = (v - vhi.astype(_F32)).astype(_BF16)
    wouthi = vhi.reshape(NK, 128).T.reshape(-1)     # [512] row-major [128,4]
    woutlo = vlo.reshape(NK, 128).T.reshape(-1)
    wbl_g = np.empty((NCORES * WBLR, H), _BF16)
    wg = wbl_g.reshape(NCORES, WBLR, H)
    for ci in range(NCORES):
        wg[ci, :WSH] = wblob[ci * WSH:(ci + 1) * WSH]
        wg[ci, WSH] = wouthi
        wg[ci, WSH + 1] = woutlo

    # --- fbl: [8*32, 512] f32 ---
    # rows 0..17 per core: shard of W_in.T [144, 512]; rows 18..30: the 13
    # LIF constant tiles ([128, 4] flattened row-major); row 31: oconst.
    consts = {}
    for li in range(3):
        beta = betas[li].astype(np.float64)
        thr = thrs[li].astype(np.float64)
        if li == 0:
            rs = np.zeros(H, np.float64)
            b = np.zeros(H, np.float64)             # b_in lives inside h_in
        else:
            w2 = W_h[li - 1].astype(np.float64) * 0.5
            rs = w2.sum(axis=1)
            b = b_h[li - 1].astype(np.float64)
        c = rs + b + thr * (beta - 1.0) - 0.5 * thr
        denom = beta - 1.0
        assert np.all(np.abs(denom) > 1e-6), "beta == 1 breaks the k-shift"
        k = -c / denom
        if li == 0:
            c0 = b_in.astype(np.float64) - thr - k  # fused with h_in psum
        else:
            c0 = rs + b - thr - k
        consts[f"c0_{li}"] = _cols(c0)
        consts[f"beta_{li}"] = _cols(beta)
        consts[f"nth2_{li}"] = _cols(-0.5 * thr)
        consts[f"kk_{li}"] = _cols(k)
    consts["binc"] = _cols(b_in)
    crows = np.stack([consts[nme].reshape(-1) for nme in CNAMES])  # [13, 512]
    orow = np.zeros(H, _F32)
    orow[0] = _F32(0.5 * W_out[0].astype(np.float64).sum()
                   + b_out.astype(np.float64)[0])
    winT = W_in.T                                    # [144, 512] f32 view
    fbl_g = np.empty((NCORES * FBLR, H), _F32)
    fg = fbl_g.reshape(NCORES, FBLR, H)
    for ci in range(NCORES):
        fg[ci, :INSH] = winT[ci * INSH:(ci + 1) * INSH]
        fg[ci, INSH:INSH + NC_] = crows
        fg[ci, INSH + NC_] = orow

    return {"wbl": wbl_g, "fbl": fbl_g}


def _build(nc, tile, mybir, bass):
    """Emit the Tile program on `nc`."""
    dt = mybir.dt
    alu = mybir.AluOpType
    AFT = mybir.ActivationFunctionType
    ts_ = bass.ts

    d_st = nc.dram_tensor("st", [BC, S], dt.float32, kind="ExternalInput").ap()
    d_ac = nc.dram_tensor("ac", [BC, A], dt.float32, kind="ExternalInput").ap()
    d_wbl = nc.dram_tensor("wbl", [WBLR, H], dt.bfloat16,
                           kind="ExternalInput").ap()
    d_fbl = nc.dram_tensor("fbl", [FBLR, H], dt.float32,
                           kind="ExternalInput").ap()
    d_out = nc.dram_tensor("out", [1, BC], dt.float32, kind="ExternalOutput").ap()

    rg = [list(range(NCORES))]

    with tile.TileContext(nc) as tc:
        with (
            tc.tile_pool(name="dram", bufs=1, space="DRAM") as dp,
            tc.tile_pool(name="wpool", bufs=1) as wp,
            tc.tile_pool(name="xpool", bufs=2) as xp,
            tc.tile_pool(name="state", bufs=1) as sp,
            tc.tile_pool(name="tmp", bufs=4) as tp,
            tc.tile_pool(name="psum", bufs=1, space="PSUM") as pp,
        ):
            # ---- reassemble the full weight blobs on-device ----
            # Each core uploaded 1/8 of each blob over the (slow) host
            # tunnel; AllGather over NeuronLink rebuilds the full blobs.
            # W_in.T goes first: the h_in matmuls are the critical path.
            insh_b = dp.tile([INSH, H], dt.float32, name="insh_b")
            nc.gpsimd.dma_start(insh_b[:], d_fbl[0:INSH, :])
            winall = dp.tile([SA, H], dt.float32, addr_space="Shared",
                             name="winall")
            nc.gpsimd.collective_compute(
                "AllGather", mybir.AluOpType.bypass, replica_groups=rg,
                ins=[insh_b.opt()], outs=[winall.opt()])
            wsh_b = dp.tile([WSH, H], dt.bfloat16, name="wsh_b")
            nc.gpsimd.dma_start(wsh_b[:], d_wbl[0:WSH, :])
            wall = dp.tile([WROWS, H], dt.bfloat16, addr_space="Shared",
                           name="wall")
            nc.gpsimd.collective_compute(
                "AllGather", mybir.AluOpType.bypass, replica_groups=rg,
                ins=[wsh_b.opt()], outs=[wall.opt()])

            # ---- resident weights & constants ----
            winS_t = wp.tile([S, H], dt.float32, name="winS_t")
            nc.sync.dma_start(winS_t[:], winall[0:S, :])
            winA_t = wp.tile([A, H], dt.float32, name="winA_t")
            nc.sync.dma_start(winA_t[:], winall[S:SA, :])
            ct = {}
            for i, nme in enumerate(CNAMES):
                t_ = wp.tile([128, NJ], dt.float32, name=f"{nme}_t")
                nc.sync.dma_start(t_[:], d_fbl[INSH + i:INSH + i + 1, :])
                ct[nme] = t_
            oconst_t = wp.tile([1, 1], dt.float32, name="oconst_t")
            nc.sync.dma_start(oconst_t[:], d_fbl[INSH + NC_:INSH + NC_ + 1, 0:1])
            wh = {}
            for li in range(LM1):
                for nm in ("whi", "wlo"):
                    for ki in range(NK):
                        wh[(nm, li, ki)] = wp.tile([128, H], dt.bfloat16,
                                                   name=f"{nm}{li}k{ki}")
            wouthi_t = wp.tile([128, NK], dt.bfloat16, name="wouthi_t")
            nc.sync.dma_start(wouthi_t[:], d_wbl[WSH:WSH + 1, :])
            woutlo_t = wp.tile([128, NK], dt.bfloat16, name="woutlo_t")
            nc.sync.dma_start(woutlo_t[:], d_wbl[WSH + 1:WSH + 2, :])
            out_sb = wp.tile([1, BC], dt.float32, name="out_sb")

            # Identity for PE-based [128, 128] transposes of the x tiles.
            ident = wp.tile([128, 128], dt.float32, name="ident")
            nc.gpsimd.memset(ident[:], 0.0)
            nc.gpsimd.affine_select(
                out=ident[:], in_=ident[:],
                compare_op=alu.not_equal, fill=1.0, base=0,
                pattern=[[-1, 128]], channel_multiplier=1)

            def dma_weights():
                for li in range(LM1):
                    for hl, nm in enumerate(("whi", "wlo")):
                        base = (2 * li + hl) * H
                        for ki in range(NK):
                            nc.sync.dma_start(
                                wh[(nm, li, ki)][:],
                                wall[base + ki * 128:base + (ki + 1) * 128, :])

            def col(nme, j):
                return ct[nme][:, j:j + 1]

            # ---- recurrence over chunk pairs ----
            for pair in range(NCH // 2):
                hin = [[None] * NJ for _ in range(2)]
                pt = [[[None] * NJ for _ in range(3)] for _ in range(2)]
                sg = [[[None] * NJ for _ in range(3)] for _ in range(2)]
                rate = [[None] * NJ for _ in range(2)]

                # per-chunk setup: h_in matmul + layer-0 t=0 LIF
                for s_ in range(2):
                    c = pair * 2 + s_
                    # x arrives untransposed [BC, feat]; build the [feat, BT]
                    # chunk via 128x128 PE transposes (exact data movement).
                    xs = xp.tile([S, BT], dt.float32, tag="xs", name=f"xs{c}")
                    xa = xp.tile([A, BT], dt.float32, tag="xa", name=f"xa{c}")
                    for r in range(BT // 128):
                        row0 = c * BT + r * 128
                        stt = xp.tile([128, S], dt.float32, tag="stt",
                                      name=f"stt{c}r{r}")
                        nc.sync.dma_start(stt[:], d_st[row0:row0 + 128, :])
                        pst = pp.tile([128, 128], dt.float32, tag="tps",
                                      bufs=2, name=f"pst{c}r{r}")
                        nc.tensor.transpose(pst[:], stt[:], ident[:])
                        nc.any.tensor_copy(xs[:, ts_(r, 128)], pst[:])
                        aat = xp.tile([128, A], dt.float32, tag="aat",
                                      name=f"aat{c}r{r}")
                        nc.sync.dma_start(aat[:], d_ac[row0:row0 + 128, :])
                        pat = pp.tile([A, 128], dt.float32, tag="tps",
                                      bufs=2, name=f"pat{c}r{r}")
                        nc.tensor.transpose(pat[:], aat[:], ident[:])
                        nc.any.tensor_copy(xa[:, ts_(r, 128)], pat[:])
                    for j in range(NJ):
                        ps = pp.tile([128, BT], dt.float32, tag="pre", bufs=5,
                                     name=f"hps{c}j{j}")
                        nc.tensor.matmul(ps[:], winS_t[:, ts_(j, 128)], xs[:],
                                         start=True, stop=False)
                        nc.tensor.matmul(ps[:], winA_t[:, ts_(j, 128)], xa[:],
                                         start=False, stop=True)
                        hv = sp.tile([128, BT], dt.float32,
                                     tag=f"hin{s_}{j}", name=f"hin{c}j{j}")
                        nc.vector.tensor_scalar(hv[:], ps[:], col("binc", j),
                                                None, alu.add)
                        hin[s_][j] = hv
                        p0 = sp.tile([128, BT], dt.float32,
                                     tag=f"p{s_}0{j}", name=f"p{c}l0j{j}")
                        nc.vector.tensor_scalar(p0[:], ps[:], col("c0_0", j),
                                                None, alu.add)
                        pt[s_][0][j] = p0
                        sg0 = sp.tile([128, BT], dt.bfloat16,
                                      tag=f"sg{s_}0{j}", name=f"sg{c}l0j{j}")
                        nc.scalar.activation(sg0[:], p0[:], AFT.Sign,
                                             bias=col("kk_0", j), scale=1.0)
                        sg[s_][0][j] = sg0
                        for li in range(1, 3):
                            pt[s_][li][j] = sp.tile(
                                [128, BT], dt.float32,
                                tag=f"p{s_}{li}{j}", name=f"p{c}l{li}j{j}")
                            sg[s_][li][j] = sp.tile(
                                [128, BT], dt.bfloat16,
                                tag=f"sg{s_}{li}{j}", name=f"sg{c}l{li}j{j}")
                        rate[s_][j] = sp.tile([128, BT], dt.bfloat16,
                                              tag=f"rate{s_}{j}",
                                              name=f"rate{c}j{j}")

                def lif_update(s_, li, j, t, pre_ap, pre_is_psum):
                    """Common LIF ops for one [128, BT] tile."""
                    c = pair * 2 + s_
                    p_ = pt[s_][li][j]
                    if t == 0:
                        nc.vector.tensor_scalar(p_[:], pre_ap,
                                                col(f"c0_{li}", j), None,
                                                alu.add)
                    else:
                        u = tp.tile([128, BT], dt.float32, tag=f"u{s_}",
                                    name=f"u{c}l{li}j{j}t{t}")
                        nc.vector.scalar_tensor_tensor(
                            u[:], p_[:], col(f"beta_{li}", j), pre_ap,
                            op0=alu.mult, op1=alu.add)
                        tau = tp.tile([128, BT], dt.float32, tag=f"tau{s_}",
                                      name=f"tau{c}l{li}j{j}t{t}")
                        nc.vector.tensor_scalar(tau[:], sg[s_][li][j][:],
                                                col(f"nth2_{li}", j), None,
                                                alu.mult)
                        nc.gpsimd.tensor_tensor(p_[:], u[:], tau[:], op=alu.add)
                    nc.scalar.activation(sg[s_][li][j][:], p_[:], AFT.Sign,
                                         bias=col(f"kk_{li}", j), scale=1.0)
                    if li == 2:
                        if t == 0:
                            nc.vector.tensor_copy(rate[s_][j][:],
                                                  sg[s_][li][j][:])
                        else:
                            nc.vector.tensor_tensor(rate[s_][j][:],
                                                    rate[s_][j][:],
                                                    sg[s_][li][j][:],
                                                    op=alu.add)

                if pair == 0:
                    dma_weights()     # bulk weights after critical xT DMAs

                def hidden_layer(s_, li, t):
                    c = pair * 2 + s_
                    for j in range(NJ):
                        ps = pp.tile([128, BT], dt.float32, tag="pre",
                                     bufs=5, name=f"ps{c}l{li}j{j}t{t}")
                        for ki in range(NK):
                            nc.tensor.matmul(
                                ps[:],
                                wh[("whi", li - 1, ki)][:, ts_(j, 128)],
                                sg[s_][li - 1][ki][:],
                                start=(ki == 0), stop=False)
                        for ki in range(NK):
                            nc.tensor.matmul(
                                ps[:],
                                wh[("wlo", li - 1, ki)][:, ts_(j, 128)],
                                sg[s_][li - 1][ki][:],
                                start=False, stop=(ki == NK - 1))
                        lif_update(s_, li, j, t, ps[:], True)

                # Interleave the two chunks at layer granularity: while
                # chunk A's layer-l LIF tail (DVE/Pool/ACT) produces its
                # sigma tiles, the PE runs chunk B's matmuls. Layer-0's
                # step-(t+1) LIF is emitted mid-step so it hides behind the
                # layer-2 matmul windows.
                for t in range(T):
                    for s_ in range(2):
                        hidden_layer(s_, 1, t)
                    if t < T - 1:
                        for s_ in range(2):
                            for j in range(NJ):
                                lif_update(s_, 0, j, t + 1, hin[s_][j][:],
                                           False)
                    for s_ in range(2):
                        hidden_layer(s_, 2, t)

                # readout per chunk
                for s_ in range(2):
                    c = pair * 2 + s_
                    ro = pp.tile([1, BT], dt.float32, tag="ro", bufs=1,
                                 name=f"ro{c}")
                    first = True
                    for wt in (wouthi_t, woutlo_t):
                        for ki in range(NK):
                            nc.tensor.matmul(ro[:], wt[:, ki:ki + 1],
                                             rate[s_][ki][:],
                                             start=first,
                                             stop=(wt is woutlo_t
                                                   and ki == NK - 1))
                            first = False
                    nc.vector.tensor_scalar(out_sb[0:1, ts_(c, BT)], ro[:],
                                            1.0 / T, oconst_t[0:1, 0:1],
                                            alu.mult, alu.add)

            nc.sync.dma_start(d_out[:], out_sb[:])
    return d_out


_CACHED = {}


def _get_compiled():
    if "nc" in _CACHED:
        return _CACHED["nc"]
    import concourse.bacc as bacc
    import concourse.bass as bass
    import concourse.tile as tile
    import concourse.mybir as mybir

    nc = bacc.Bacc("TRN2", target_bir_lowering=False, debug=False,
                   num_devices=NCORES)
    _build(nc, tile, mybir, bass)
    nc.compile()
    _CACHED["nc"] = nc
    return nc


def _get_runner():
    """Build (once) a cached jitted SPMD executable for `nc`.

    Mirrors concourse.bass2jax.run_bass_via_pjrt, but holds on to the
    jitted shard_map callable: rebuilding it per call costs ~450 ms of
    jax re-tracing for identical machine code.
    """
    if "runner" in _CACHED:
        return _CACHED["runner"]

    import jax
    from jax.sharding import Mesh, PartitionSpec
    from jax.experimental.shard_map import shard_map
    from concourse import bass2jax, mybir
    from concourse.bass2jax import _bass_exec_p, install_neuronx_cc_hook

    nc = _get_compiled()
    install_neuronx_cc_hook()

    partition_name = (nc.partition_id_tensor.name
                      if nc.partition_id_tensor else None)
    in_names, out_names, out_avals, zero_shapes = [], [], [], []
    for alloc in nc.m.functions[0].allocations:
        if not isinstance(alloc, mybir.MemoryLocationSet):
            continue
        name = alloc.memorylocations[0].name
        if alloc.kind == "ExternalInput":
            if name != partition_name:
                in_names.append(name)
        elif alloc.kind == "ExternalOutput":
            out_names.append(name)
            shape = tuple(alloc.tensor_shape)
            dtype = mybir.dt.np(alloc.dtype)
            out_avals.append(jax.core.ShapedArray(shape, dtype))
            zero_shapes.append((shape, dtype))
    n_params = len(in_names)
    n_outs = len(out_avals)
    in_names_full = list(in_names) + out_names
    if partition_name is not None:
        in_names_full.append(partition_name)

    def _body(*args):
        operands = list(args)
        if partition_name is not None:
            operands.append(bass2jax.partition_id_tensor())
        outs = _bass_exec_p.bind(
            *operands,
            out_avals=tuple(out_avals),
            in_names=tuple(in_names_full),
            out_names=tuple(out_names),
            lowering_input_output_aliases=(),
            sim_require_finite=True,
            sim_require_nnan=True,
            nc=nc,
        )
        return tuple(outs)

    devices = jax.devices()[:NCORES]
    assert len(devices) == NCORES
    mesh = Mesh(np.asarray(devices), ("core",))
    in_specs = (PartitionSpec("core"),) * (n_params + n_outs)
    out_specs = (PartitionSpec("core"),) * len(out_names)
    donate = tuple(range(n_params, n_params + n_outs))
    sharded = jax.jit(
        shard_map(_body, mesh=mesh, in_specs=in_specs, out_specs=out_specs,
                  check_rep=False),
        donate_argnums=donate, keep_unused=True,
    )

    # Weight blobs are parameters: keep a device-resident copy across calls
    # (content-hash verified; any change re-uploads).  The batch tensor xt
    # is per-call data and always ships.
    import hashlib
    from jax.sharding import NamedSharding
    core_sharding = NamedSharding(mesh, PartitionSpec("core"))
    wcache = {}

    def _dev(name, arr):
        arr = np.ascontiguousarray(arr)
        hit = wcache.get(name)
        if hit is not None and hit[2] is arr:      # same host-cached blob
            return hit[1]
        digest = hashlib.blake2b(arr, digest_size=16).digest()
        if hit is not None and hit[0] == digest:
            return hit[1]
        dev = jax.device_put(arr, core_sharding)
        wcache[name] = (digest, dev, arr)
        return dev

    def runner(global_in):
        # st/ac are per-call batch data and always ship; the weight blobs
        # stay device-resident while their content hash is unchanged.
        args = [global_in[nme] if nme in ("st", "ac")
                else _dev(nme, global_in[nme])
                for nme in in_names]
        zeros = [np.zeros((NCORES * shp[0], *shp[1:]), dtp)
                 for shp, dtp in zero_shapes]
        out_arrs = sharded(*args, *zeros)
        return {nme: np.asarray(out_arrs[i])
                for i, nme in enumerate(out_names)}

    _CACHED["runner"] = runner
    return runner


def run(inputs, trace=False, trace_kwargs=None):
    """Build+run; returns (output [B,1] f32, raw out dict)."""
    global_in = _prepare_host(inputs)
    runner = _get_runner()
    outs = runner(global_in)
    out = outs["out"].reshape(B, 1)
    return np.ascontiguousarray(out, dtype=_F32), outs


def kernel(**inputs):
    out, _ = run(inputs, trace=False)
    return out
